# revision 1
# baseline (speedup 1.0000x reference)
"""Trainium2 Bass kernel for nn_LocalizedFiltering (fused cat-conv2d x2 + residual + RMSNorm).

Strategy: sequence-parallel across 8 NeuronCores (one sequence of 2048 tokens +
1 cache row per core) -- no collectives needed. On-device compute uses a
transposed (feature-on-partition) layout so the kernel-2 causal conv's
shift-add becomes a column-window offset absorbed into the matmul rhs windows.
Matmuls run in bf16 (fp32 PSUM accumulation); residual + RMSNorm in fp32.

Per core s:
  xt1T = [cache1_s ; X_s].T                     # [2048, 2049] bf16
  layer1: c = xt1T.T@W1 windows -> o1T          # [1024, 2048] -> xt2T cols 1..
  layer2: same with W2 -> o2T tiles [128,512]
  epilogue: PE-transpose to row-major, + X residual, RMSNorm, DMA out rows.
ln_weight is applied exactly on the host (out *= ln_weight).
"""

import os

import numpy as np
import ml_dtypes

BS, L, D, CACHE = 8, 2048, 2048, 64
T = BS * L
H = D // 2          # 1024
EPS = 1e-6
NCORES = 8
BLK = 512           # token block (= one PSUM bank of fp32)
NBLK = L // BLK     # 4
KT1 = D // 128      # 16 contraction tiles, layer 1
KT2 = H // 128      # 8 contraction tiles, layer 2
QT1 = H // 128      # 8 output-feature tiles, layer 1 (per half)
QT2 = D // 128      # 16 output-feature tiles, layer 2 (per half)

TRACE = bool(int(os.environ.get("BASS_KERNEL_TRACE", "0")))
LAST_EXEC_NS = None
LAST_RESULTS = None

_NC_CACHE = {}


def _build_bass():
    if "nc" in _NC_CACHE:
        return _NC_CACHE["nc"]

    import concourse.bacc as bacc
    import concourse.tile as tile
    import concourse.mybir as mybir
    from concourse.masks import make_identity

    fp32 = mybir.dt.float32
    bf16 = mybir.dt.bfloat16
    Act = mybir.ActivationFunctionType

    nc = bacc.Bacc("TRN2", target_bir_lowering=False)

    xt1 = nc.declare_dram_parameter("xt1", [D, L + 1], bf16, isOutput=False)
    xrow = nc.declare_dram_parameter("xrow", [L, D], fp32, isOutput=False)
    c2 = nc.declare_dram_parameter("c2", [H, 1], bf16, isOutput=False)
    w1 = nc.declare_dram_parameter("w1", [D, D], bf16, isOutput=False)
    w2 = nc.declare_dram_parameter("w2", [H, 2 * D], bf16, isOutput=False)
    b1 = nc.declare_dram_parameter("b1", [H, 1], fp32, isOutput=False)
    b2 = nc.declare_dram_parameter("b2", [D, 1], fp32, isOutput=False)
    out = nc.declare_dram_parameter("out", [L, D], fp32, isOutput=True)

    with tile.TileContext(nc) as tc, \
            tc.tile_pool(name="wpool", bufs=1) as wpool, \
            tc.tile_pool(name="wpre", bufs=1) as wpre, \
            tc.tile_pool(name="xt1p", bufs=2) as xt1p, \
            tc.tile_pool(name="xt2p", bufs=1) as xt2p, \
            tc.tile_pool(name="xrowp", bufs=8) as xrowp, \
            tc.tile_pool(name="rowp", bufs=4) as rowp, \
            tc.tile_pool(name="tmp", bufs=2) as tmp, \
            tc.tile_pool(name="const", bufs=1) as const, \
            tc.tile_pool(name="psmm", bufs=3, space="PSUM") as psmm, \
            tc.tile_pool(name="psdp", bufs=1, space="PSUM") as psdp, \
            tc.tile_pool(name="pstr", bufs=4, space="PSUM") as pstr:

        ident = const.tile([128, 128], fp32)
        make_identity(nc, ident)
        epssb = const.tile([128, 1], fp32)
        nc.vector.memset(epssb, EPS)

        b1sb = const.tile([128, QT1, 1], fp32)
        b2sb = const.tile([128, QT2, 1], fp32)
        xt2sb = xt2p.tile([128, KT2, L + 1], bf16)

        # ---------------- Phase A: layer 1 -> xt2T (bf16) ----------------
        # W1 as 8 pair-tiles [128, 2, D]; same slots later reused by W2 k-tiles.
        NW = KT1 // 2  # 8
        w1t = []
        x1k0 = []
        # interleave issue order: w pair j, then 2 x1 k-tiles of block 0, so the
        # first matmuls unblock after a few MB of DMA.
        for j in range(NW):
            wj = wpool.tile([128, 2, D], bf16, tag=f"w{j}", name=f"w1_{j}")
            for kk in range(2):
                nc.sync.dma_start(
                    out=wj[:, kk, :],
                    in_=w1[(2 * j + kk) * 128:(2 * j + kk + 1) * 128, :])
            w1t.append(wj)
            for kk in range(2):
                k = 2 * j + kk
                xk = xt1p.tile([128, BLK + 1], bf16, tag=f"x1k{k}", name=f"x1_0_{k}")
                nc.sync.dma_start(
                    out=xk, in_=xt1[k * 128:(k + 1) * 128, 0:BLK + 1])
                x1k0.append(xk)

        nc.sync.dma_start(
            out=b1sb, in_=b1.rearrange("(q p) o -> p q o", p=128))
        nc.sync.dma_start(
            out=b2sb, in_=b2.rearrange("(q p) o -> p q o", p=128))
        nc.sync.dma_start(
            out=xt2sb[:, :, 0:1], in_=c2.rearrange("(k p) o -> p k o", p=128))

        for b in range(NBLK):
            if b == 0:
                x1k = x1k0
            else:
                x1k = []
                for k in range(KT1):
                    xk = xt1p.tile([128, BLK + 1], bf16, tag=f"x1k{k}",
                                   name=f"x1_{b}_{k}")
                    nc.sync.dma_start(
                        out=xk,
                        in_=xt1[k * 128:(k + 1) * 128, b * BLK:b * BLK + BLK + 1])
                    x1k.append(xk)
            # k-outer over 8 concurrent psum groups: PE consumes each W1 pair
            # as it lands (startup), and frees W1 slots progressively on the
            # last block so the W2 stream overlaps the tail of phase A.
            psA = []
            for q in range(QT1):
                if q < 3:
                    ps = psmm.tile([128, BLK], fp32, tag="mm", name=f"psA_{b}_{q}")
                elif q < 7:
                    ps = pstr.tile([128, BLK], fp32, tag="pt", name=f"psA_{b}_{q}")
                else:
                    ps = psdp.tile([128, BLK], fp32, tag="dump", name=f"psA_{b}_{q}")
                psA.append(ps)
            for k in range(KT1):
                for q in range(QT1):
                    nc.tensor.matmul(
                        psA[q], lhsT=w1t[k // 2][:, k % 2, q * 128:(q + 1) * 128],
                        rhs=x1k[k][:, 0:BLK],
                        start=(k == 0), stop=False)
                    nc.tensor.matmul(
                        psA[q],
                        lhsT=w1t[k // 2][:, k % 2, H + q * 128:H + (q + 1) * 128],
                        rhs=x1k[k][:, 1:BLK + 1],
                        start=False, stop=(k == KT1 - 1))
            for q in range(QT1):
                nc.scalar.activation(
                    out=xt2sb[:, q, 1 + b * BLK:1 + (b + 1) * BLK], in_=psA[q],
                    func=Act.Identity, bias=b1sb[:, q, :], scale=1.0)

        # ---------------- Phase B: layer 2 + residual + RMSNorm ----------------
        # W2 k-tiles: k=0,1 prefetched into dedicated slots; k>=2 reuse w slots.
        w2t = []
        for k in range(KT2):
            if k < 2:
                wk = wpre.tile([128, 2 * D], bf16, tag=f"wp{k}", name=f"w2_{k}")
            else:
                wk = wpool.tile([128, 2 * D], bf16, tag=f"w{k - 2}", name=f"w2_{k}")
            nc.sync.dma_start(out=wk, in_=w2[k * 128:(k + 1) * 128, :])
            w2t.append(wk)

        for b in range(NBLK):
            rowcs = []
            accs = []
            xrcs = {}

            def load_xr_group(c4, b=b, xrcs=xrcs):
                sl = slice(c4 * BLK, (c4 + 1) * BLK)
                for c in range(4):
                    t = xrowp.tile([128, BLK], fp32, tag="xrc",
                                   name=f"xrc_{b}_{c}_{c4}")
                    r0 = b * BLK + c * 128
                    nc.sync.dma_start(out=t, in_=xrow[r0:r0 + 128, sl])
                    xrcs[(c, c4)] = t

            for c in range(4):
                rowcs.append(rowp.tile([128, D], fp32, tag="rowc", name=f"rowc_{b}_{c}"))
                accs.append(tmp.tile([128, 4], fp32, tag=f"acc4_{c}",
                                     name=f"acc4_{b}_{c}"))
            load_xr_group(0)
            for q in range(QT2):
                ps = psmm.tile([128, BLK], fp32, tag="mm", name=f"psB_{b}_{q}")
                for k in range(KT2):
                    nc.tensor.matmul(
                        ps, lhsT=w2t[k][:, q * 128:(q + 1) * 128],
                        rhs=xt2sb[:, k, b * BLK:(b + 1) * BLK],
                        start=(k == 0), stop=False)
                for k in range(KT2):
                    nc.tensor.matmul(
                        ps, lhsT=w2t[k][:, D + q * 128:D + (q + 1) * 128],
                        rhs=xt2sb[:, k, b * BLK + 1:(b + 1) * BLK + 1],
                        start=False, stop=(k == KT2 - 1))
                o2q = tmp.tile([128, BLK], fp32, tag="o2q", name=f"o2q_{b}_{q}")
                nc.scalar.activation(
                    out=o2q, in_=ps,
                    func=Act.Identity, bias=b2sb[:, q, :], scale=1.0)
                for c in range(4):
                    pt = pstr.tile([128, 128], fp32, tag="pt", name=f"pt_{b}_{q}_{c}")
                    nc.tensor.transpose(pt, o2q[:, c * 128:(c + 1) * 128], ident)
                    nc.vector.tensor_copy(
                        out=rowcs[c][:, q * 128:(q + 1) * 128], in_=pt)
                if q % 4 == 3:
                    # column group c4 = q//4 (cols c4*512 .. +512) complete for
                    # every chunk: fold residual + partial sum-of-squares now so
                    # almost no norm work remains after the last matmul.
                    c4 = q // 4
                    sl = slice(c4 * BLK, (c4 + 1) * BLK)
                    if c4 < 3:
                        load_xr_group(c4 + 1)
                    for c in range(4):
                        nc.vector.tensor_add(
                            out=rowcs[c][:, sl], in0=rowcs[c][:, sl],
                            in1=xrcs[(c, c4)])
                        dump = psdp.tile([128, BLK], fp32, tag="dump",
                                         name=f"dump_{b}_{c}_{c4}")
                        nc.scalar.activation(
                            out=dump, in_=rowcs[c][:, sl],
                            func=Act.Square, accum_out=accs[c][:, c4:c4 + 1])
            # finalize per 128-token chunk: rstd + scale + store
            for c in range(4):
                tok0 = b * BLK + c * 128
                rstd = tmp.tile([128, 1], fp32, tag="rstd", name=f"rstd_{b}_{c}")
                nc.vector.tensor_reduce(
                    out=rstd, in_=accs[c], axis=mybir.AxisListType.X,
                    op=mybir.AluOpType.add)
                nc.scalar.activation(
                    out=rstd, in_=rstd,
                    func=Act.Sqrt, bias=epssb, scale=1.0 / D)
                nc.vector.reciprocal(out=rstd, in_=rstd)
                if c % 2 == 0:
                    nc.scalar.activation(
                        out=rowcs[c], in_=rowcs[c],
                        func=Act.Identity, bias=0.0, scale=rstd)
                else:
                    nc.vector.tensor_scalar_mul(
                        out=rowcs[c], in0=rowcs[c], scalar1=rstd)
                nc.sync.dma_start(out=out[tok0:tok0 + 128, :], in_=rowcs[c])

    nc.finalize()
    _NC_CACHE["nc"] = nc
    return nc


def _np_reference(inputs, pre_lf_indexs, out_lf_indexs, input_lf_loc, out_lf_loc,
                  inputs_loc, outputs_loc, lf1_caches, lf2_caches,
                  conv1_weight, conv2_weight, conv1_bias, conv2_bias, ln_weight):
    """Generic numpy fallback (only used if the index structure is unexpected)."""
    def fused(x, cache, pre_idx, in_lf_loc, in_loc, out_loc, W):
        bs = pre_idx.shape[0]
        xt = np.zeros((x.shape[0] + bs, x.shape[1]), x.dtype)
        xt[in_loc] = x
        xt[in_lf_loc] = cache[pre_idx]
        c = xt @ W
        h = c.shape[1] // 2
        y = c[:-1, :h] + c[1:, h:]
        return y[out_loc]

    o1 = fused(inputs, lf1_caches, pre_lf_indexs, input_lf_loc,
               inputs_loc, outputs_loc, conv1_weight) + conv1_bias
    o2 = fused(o1, lf2_caches, pre_lf_indexs, input_lf_loc,
               inputs_loc, outputs_loc, conv2_weight) + conv2_bias
    o3 = o2 + inputs
    var = np.mean(o3 * o3, axis=-1, keepdims=True)
    return (o3 / np.sqrt(var + EPS) * ln_weight).astype(np.float32)


def kernel(**inputs):
    global LAST_EXEC_NS, LAST_RESULTS
    inp = {k: np.asarray(v) for k, v in inputs.items()}
    x = inp["inputs"].astype(np.float32, copy=False)
    lnw = inp["ln_weight"].astype(np.float32, copy=False)

    s = np.arange(BS, dtype=np.int64)
    j = np.arange(L, dtype=np.int64)
    structured = (
        np.array_equal(inp["inputs_loc"], (s[:, None] * (L + 1) + 1 + j[None, :]).reshape(-1))
        and np.array_equal(inp["outputs_loc"], (s[:, None] * (L + 1) + j[None, :]).reshape(-1))
        and np.array_equal(inp["input_lf_loc"], s * (L + 1))
    )
    if not structured:
        return _np_reference(**inp)

    from concourse.bass_utils import run_bass_kernel_spmd

    nc = _build_bass()

    bf16 = ml_dtypes.bfloat16
    pre_idx = inp["pre_lf_indexs"].astype(np.int64)
    w1b = np.ascontiguousarray(inp["conv1_weight"].astype(bf16))
    w2b = np.ascontiguousarray(inp["conv2_weight"].astype(bf16))
    b1f = np.ascontiguousarray(inp["conv1_bias"].astype(np.float32).reshape(H, 1))
    b2f = np.ascontiguousarray(inp["conv2_bias"].astype(np.float32).reshape(D, 1))

    in_maps = []
    for sq in range(BS):
        xs = x[sq * L:(sq + 1) * L]                       # [2048, 2048]
        a = np.empty((D, L + 1), np.float32)
        a[:, 0] = inp["lf1_caches"][pre_idx[sq]]
        a[:, 1:] = xs.T
        in_maps.append({
            "xt1": np.ascontiguousarray(a.astype(bf16)),
            "xrow": np.ascontiguousarray(xs),
            "c2": np.ascontiguousarray(
                inp["lf2_caches"][pre_idx[sq]].astype(bf16).reshape(H, 1)),
            "w1": w1b,
            "w2": w2b,
            "b1": b1f,
            "b2": b2f,
        })

    res = run_bass_kernel_spmd(nc, in_maps, list(range(NCORES)), trace=TRACE)
    LAST_EXEC_NS = res.exec_time_ns
    LAST_RESULTS = res
    out = np.concatenate([res.results[i]["out"] for i in range(NCORES)], axis=0)
    if not np.all(lnw == 1.0):
        out = out * lnw[None, :]
    return out.astype(np.float32)



# revision 7
# speedup vs baseline: 1.1114x; 1.1114x over previous
"""Trainium2 Bass kernel for nn_LocalizedFiltering (fused cat-conv2d x2 + residual + RMSNorm).

Strategy: sequence-parallel across 8 NeuronCores (one sequence of 2048 tokens +
1 cache row per core) -- no collectives needed.

Layer 1 runs feature-on-partition (output o1T = [1024 feat, 2049 tok] bf16 in
SBUF); layer 2 runs token-on-partition (lhsT = o1T column windows, rhs = W2
rows), so its PSUM output is already row-major [128 tok, 512 feat] -- no PE
transposes, and the residual + RMSNorm epilogue works directly on token rows.
The kernel-2 causal conv's shift-add is absorbed as two accumulated matmul
windows in both layers. conv2_bias is folded into the residual on the host
(xrb2 = x + b2); ln_weight is applied exactly on the host.

Matmuls run in bf16 (fp32 PSUM accumulation); epilogue in fp32.
"""

import os

import numpy as np
import ml_dtypes

BS, L, D, CACHE = 8, 2048, 2048, 64
T = BS * L
H = D // 2          # 1024
EPS = 1e-6
NCORES = 8
BLK = 512           # token block (= one PSUM bank of fp32)
NBLK = L // BLK     # 4
KT1 = D // 128      # 16 contraction tiles, layer 1
KT2 = H // 128      # 8 contraction tiles, layer 2
QT1 = H // 128      # 8 output-feature tiles, layer 1 (per half)
TT = L // 128       # 16 token tiles, layer 2

TRACE = bool(int(os.environ.get("BASS_KERNEL_TRACE", "0")))
LAST_EXEC_NS = None
LAST_RESULTS = None

_NC_CACHE = {}


def _build_bass():
    if "nc" in _NC_CACHE:
        return _NC_CACHE["nc"]

    import concourse.bacc as bacc
    import concourse.tile as tile
    import concourse.mybir as mybir

    fp32 = mybir.dt.float32
    bf16 = mybir.dt.bfloat16
    Act = mybir.ActivationFunctionType

    nc = bacc.Bacc("TRN2", target_bir_lowering=False)

    xt1 = nc.declare_dram_parameter("xt1", [D, L + 1], bf16, isOutput=False)
    xrb2 = nc.declare_dram_parameter("xrb2", [L, D], fp32, isOutput=False)
    c2 = nc.declare_dram_parameter("c2", [H, 1], bf16, isOutput=False)
    w1 = nc.declare_dram_parameter("w1", [D, D], bf16, isOutput=False)
    w2 = nc.declare_dram_parameter("w2", [H, 2 * D], bf16, isOutput=False)
    b1 = nc.declare_dram_parameter("b1", [H, 1], fp32, isOutput=False)
    out = nc.declare_dram_parameter("out", [L, D], fp32, isOutput=True)

    with tile.TileContext(nc) as tc, \
            tc.tile_pool(name="w1p", bufs=1) as w1p, \
            tc.tile_pool(name="w2p", bufs=1) as w2p, \
            tc.tile_pool(name="xt2p", bufs=1) as xt2p, \
            tc.tile_pool(name="x1p", bufs=1) as x1p, \
            tc.tile_pool(name="xrp", bufs=1) as xrp, \
            tc.tile_pool(name="rowp", bufs=2) as rowp, \
            tc.tile_pool(name="tmp", bufs=2) as tmp, \
            tc.tile_pool(name="const", bufs=1) as const, \
            tc.tile_pool(name="ps", bufs=1, space="PSUM") as psp:

        # PE clock warmup input: memset first so the dummy matmuls can start
        # as early as possible (the tensor engine ramps 0.65->1.2->2.4 GHz
        # with ~3us of sustained work; the first real matmul can't start
        # before its DMAs land at ~3.6us).
        n_warm = int(os.environ.get("BASS_WARMUP_MM", "14"))
        warm_free = int(os.environ.get("BASS_WARMUP_FREE", "256"))
        dum = const.tile([128, BLK], bf16)
        nc.vector.memset(dum, 0.0)

        # ------------- startup DMA stream (issue order == transfer order) ----
        # x1 tile (b0,k0) first, then W1 pair 0 in half-row pieces so the first
        # matmul unblocks after ~1.2 MB of DMA.
        x1k = []
        for k in range(KT1):
            x1k.append(x1p.tile([128, BLK + 1], bf16, tag=f"x1_{k}",
                                name=f"x1_0_{k}"))
        w1t = []
        for j in range(KT1 // 2):
            w1t.append(w1p.tile([128, 2, D], bf16, tag=f"w1_{j}",
                                name=f"w1_{j}"))

        nc.sync.dma_start(out=x1k[0], in_=xt1[0:128, 0:BLK + 1])
        nc.sync.dma_start(out=w1t[0][:, 0, 0:BLK], in_=w1[0:128, 0:BLK])
        nc.sync.dma_start(out=w1t[0][:, 0, BLK:D], in_=w1[0:128, BLK:D])
        nc.sync.dma_start(out=x1k[1], in_=xt1[128:256, 0:BLK + 1])
        nc.sync.dma_start(out=w1t[0][:, 1, :], in_=w1[128:256, :])
        nc.sync.dma_start(out=x1k[2], in_=xt1[256:384, 0:BLK + 1])
        for j in range(1, KT1 // 2):
            for kk in range(2):
                nc.sync.dma_start(
                    out=w1t[j][:, kk, :],
                    in_=w1[(2 * j + kk) * 128:(2 * j + kk + 1) * 128, :])
            for k in (2 * j + 1, 2 * j + 2):
                if k < KT1:
                    nc.sync.dma_start(
                        out=x1k[k], in_=xt1[k * 128:(k + 1) * 128, 0:BLK + 1])

        b1sb = const.tile([128, QT1, 1], fp32)
        epssb = const.tile([128, 1], fp32)
        xt2sb = xt2p.tile([128, KT2, L + 1], bf16)
        sqdump = const.tile([128, BLK], fp32)
        nc.sync.dma_start(out=b1sb, in_=b1.rearrange("(q p) o -> p q o", p=128))
        nc.sync.dma_start(
            out=xt2sb[:, :, 0:1], in_=c2.rearrange("(k p) o -> p k o", p=128))
        nc.vector.memset(epssb, EPS)

        # PE clock warmup: throwaway matmuls on the memset tile while the
        # first weight/activation DMAs are in flight, so the real matmuls
        # start at full clock.  Results go to a PSUM bank that the first real
        # accumulation group overwrites (start=True).
        if n_warm:
            wps = psp.tile([128, BLK], fp32, tag="ps0", name="ps_warm")
            for i in range(n_warm):
                nc.tensor.matmul(
                    wps[:, 0:warm_free], lhsT=dum[:, 0:128],
                    rhs=dum[:, 0:warm_free],
                    start=(i == 0), stop=(i == n_warm - 1))

        w2t = []
        for k in range(KT2):
            w2t.append(w2p.tile([128, 2 * D], bf16, tag=f"w2_{k}",
                                name=f"w2_{k}"))
        xrt = []
        for j in range(4):
            xrt.append(xrp.tile([128, BLK], fp32, tag=f"xr_{j}",
                                name=f"xr_0_{j}"))

        # ---------------- Phase A: layer 1 -> xt2sb (o1T, bf16) --------------
        for b in range(NBLK):
            psA = [psp.tile([128, BLK], fp32, tag=f"ps{q}", name=f"psA_{b}_{q}")
                   for q in range(QT1)]
            for k in range(KT1):
                xk = x1k[k]
                for q in range(QT1):
                    nc.tensor.matmul(
                        psA[q], lhsT=w1t[k // 2][:, k % 2, q * 128:(q + 1) * 128],
                        rhs=xk[:, 0:BLK], start=(k == 0), stop=False)
                for q in range(QT1):
                    nc.tensor.matmul(
                        psA[q],
                        lhsT=w1t[k // 2][:, k % 2, H + q * 128:H + (q + 1) * 128],
                        rhs=xk[:, 1:BLK + 1], start=False, stop=(k == KT1 - 1))
                if b < NBLK - 1:
                    # refresh this k-slot for the next block (WAR dep on the
                    # 16 matmuls just issued -- already satisfied when the DMA
                    # reaches the head of the queue).
                    x1k[k] = x1p.tile([128, BLK + 1], bf16, tag=f"x1_{k}",
                                      name=f"x1_{b + 1}_{k}")
                    nc.sync.dma_start(
                        out=x1k[k],
                        in_=xt1[k * 128:(k + 1) * 128,
                                (b + 1) * BLK:(b + 1) * BLK + BLK + 1])
            for q in range(QT1):
                nc.scalar.activation(
                    out=xt2sb[:, q, 1 + b * BLK:1 + (b + 1) * BLK], in_=psA[q],
                    func=Act.Identity, bias=b1sb[:, q, :], scale=1.0)
            # stagger W2 loads across blocks 0..2 so they never gate phase B
            for k in {0: (0, 1, 2), 1: (3, 4, 5), 2: (6, 7)}.get(b, ()):
                nc.sync.dma_start(out=w2t[k], in_=w2[k * 128:(k + 1) * 128, :])
            if b == NBLK - 1:
                # first token-tile's residual chunks for phase B
                for j in range(4):
                    nc.sync.dma_start(
                        out=xrt[j], in_=xrb2[0:128, j * BLK:(j + 1) * BLK])

        # ---------- Phase B: layer 2 token-major + residual + RMSNorm --------
        for t in range(TT):
            t0 = t * 128
            last = t == TT - 1
            rowc = rowp.tile([128, D], fp32, tag="row", name=f"row_{t}")
            # final tile: the last feature chunk is only 128 wide so the
            # add/square on the rstd critical path after the very last matmul
            # is short.
            chunks = [(0, BLK), (BLK, BLK), (2 * BLK, BLK),
                      (3 * BLK, 384), (3 * BLK + 384, 128)] if last else \
                     [(0, BLK), (BLK, BLK), (2 * BLK, BLK), (3 * BLK, BLK)]
            acc = tmp.tile([128, len(chunks)], fp32, tag="acc", name=f"acc_{t}")
            for ci, (c0, cw) in enumerate(chunks):
                f4 = c0 // BLK
                ps = psp.tile([128, BLK], fp32, tag=f"ps{(t * 4 + ci) % 8}",
                              name=f"psB_{t}_{ci}")
                for k in range(KT2):
                    nc.tensor.matmul(
                        ps[:, 0:cw], lhsT=xt2sb[:, k, t0:t0 + 128],
                        rhs=w2t[k][:, c0:c0 + cw],
                        start=(k == 0), stop=False)
                for k in range(KT2):
                    nc.tensor.matmul(
                        ps[:, 0:cw], lhsT=xt2sb[:, k, t0 + 1:t0 + 129],
                        rhs=w2t[k][:, D + c0:D + c0 + cw],
                        start=False, stop=(k == KT2 - 1))
                sl = slice(c0, c0 + cw)
                nc.vector.tensor_add(out=rowc[:, sl], in0=ps[:, 0:cw],
                                     in1=xrt[f4][:, c0 - f4 * BLK:c0 - f4 * BLK + cw])
                if not last and ci < 4:
                    xrt[ci] = xrp.tile([128, BLK], fp32, tag=f"xr_{ci}",
                                       name=f"xr_{t + 1}_{ci}")
                    nc.sync.dma_start(
                        out=xrt[ci],
                        in_=xrb2[t0 + 128:t0 + 256, ci * BLK:(ci + 1) * BLK])
                nc.scalar.activation(
                    out=sqdump[:, 0:cw], in_=rowc[:, sl],
                    func=Act.Square, accum_out=acc[:, ci:ci + 1])
            rstd = tmp.tile([128, 1], fp32, tag="rstd", name=f"rstd_{t}")
            nc.vector.tensor_reduce(
                out=rstd, in_=acc, axis=mybir.AxisListType.X,
                op=mybir.AluOpType.add)
            nc.scalar.activation(
                out=rstd, in_=rstd, func=Act.Sqrt, bias=epssb, scale=1.0 / D)
            nc.vector.reciprocal(out=rstd, in_=rstd)
            if last:
                # 4-way scale/store, DVE first, so the first store's
                # descriptors enter the (serialized) DMA path asap.
                for j in range(4):
                    sl = slice(j * BLK, (j + 1) * BLK)
                    if j % 2 == 0:
                        nc.vector.tensor_scalar_mul(
                            out=rowc[:, sl], in0=rowc[:, sl], scalar1=rstd)
                    else:
                        nc.scalar.activation(
                            out=rowc[:, sl], in_=rowc[:, sl],
                            func=Act.Identity, bias=0.0, scale=rstd)
                    nc.sync.dma_start(
                        out=out[t0:t0 + 128, sl], in_=rowc[:, sl])
            else:
                nc.scalar.activation(
                    out=rowc[:, 0:H], in_=rowc[:, 0:H],
                    func=Act.Identity, bias=0.0, scale=rstd)
                nc.vector.tensor_scalar_mul(
                    out=rowc[:, H:D], in0=rowc[:, H:D], scalar1=rstd)
                nc.sync.dma_start(out=out[t0:t0 + 128, 0:H], in_=rowc[:, 0:H])
                nc.sync.dma_start(out=out[t0:t0 + 128, H:D], in_=rowc[:, H:D])

    nc.finalize()
    _NC_CACHE["nc"] = nc
    return nc


def _np_reference(inputs, pre_lf_indexs, out_lf_indexs, input_lf_loc, out_lf_loc,
                  inputs_loc, outputs_loc, lf1_caches, lf2_caches,
                  conv1_weight, conv2_weight, conv1_bias, conv2_bias, ln_weight):
    """Generic numpy fallback (only used if the index structure is unexpected)."""
    def fused(x, cache, pre_idx, in_lf_loc, in_loc, out_loc, W):
        bs = pre_idx.shape[0]
        xt = np.zeros((x.shape[0] + bs, x.shape[1]), x.dtype)
        xt[in_loc] = x
        xt[in_lf_loc] = cache[pre_idx]
        c = xt @ W
        h = c.shape[1] // 2
        y = c[:-1, :h] + c[1:, h:]
        return y[out_loc]

    o1 = fused(inputs, lf1_caches, pre_lf_indexs, input_lf_loc,
               inputs_loc, outputs_loc, conv1_weight) + conv1_bias
    o2 = fused(o1, lf2_caches, pre_lf_indexs, input_lf_loc,
               inputs_loc, outputs_loc, conv2_weight) + conv2_bias
    o3 = o2 + inputs
    var = np.mean(o3 * o3, axis=-1, keepdims=True)
    return (o3 / np.sqrt(var + EPS) * ln_weight).astype(np.float32)


def kernel(**inputs):
    global LAST_EXEC_NS, LAST_RESULTS
    inp = {k: np.asarray(v) for k, v in inputs.items()}
    x = inp["inputs"].astype(np.float32, copy=False)
    lnw = inp["ln_weight"].astype(np.float32, copy=False)

    s = np.arange(BS, dtype=np.int64)
    j = np.arange(L, dtype=np.int64)
    structured = (
        np.array_equal(inp["inputs_loc"], (s[:, None] * (L + 1) + 1 + j[None, :]).reshape(-1))
        and np.array_equal(inp["outputs_loc"], (s[:, None] * (L + 1) + j[None, :]).reshape(-1))
        and np.array_equal(inp["input_lf_loc"], s * (L + 1))
    )
    if not structured:
        return _np_reference(**inp)

    from concourse.bass_utils import run_bass_kernel_spmd

    nc = _build_bass()

    bf16 = ml_dtypes.bfloat16
    pre_idx = inp["pre_lf_indexs"].astype(np.int64)
    w1b = np.ascontiguousarray(inp["conv1_weight"].astype(bf16))
    w2b = np.ascontiguousarray(inp["conv2_weight"].astype(bf16))
    b1f = np.ascontiguousarray(inp["conv1_bias"].astype(np.float32).reshape(H, 1))
    b2row = inp["conv2_bias"].astype(np.float32).reshape(1, D)

    in_maps = []
    for sq in range(BS):
        xs = x[sq * L:(sq + 1) * L]                       # [2048, 2048]
        a = np.empty((D, L + 1), np.float32)
        a[:, 0] = inp["lf1_caches"][pre_idx[sq]]
        a[:, 1:] = xs.T
        in_maps.append({
            "xt1": np.ascontiguousarray(a.astype(bf16)),
            "xrb2": np.ascontiguousarray(xs + b2row),
            "c2": np.ascontiguousarray(
                inp["lf2_caches"][pre_idx[sq]].astype(bf16).reshape(H, 1)),
            "w1": w1b,
            "w2": w2b,
            "b1": b1f,
        })

    res = run_bass_kernel_spmd(nc, in_maps, list(range(NCORES)), trace=TRACE)
    LAST_EXEC_NS = res.exec_time_ns
    LAST_RESULTS = res
    out = np.concatenate([res.results[i]["out"] for i in range(NCORES)], axis=0)
    if not np.all(lnw == 1.0):
        out = out * lnw[None, :]
    return out.astype(np.float32)


# revision 15
# speedup vs baseline: 1.1143x; 1.0026x over previous
"""Trainium2 Bass kernel for nn_LocalizedFiltering (fused cat-conv2d x2 + residual + RMSNorm).

Strategy: sequence-parallel across 8 NeuronCores (one sequence of 2048 tokens +
1 cache row per core) -- no collectives needed.

Layer 1 runs feature-on-partition (output o1T = [1024 feat, 2049 tok] bf16 in
SBUF); layer 2 runs token-on-partition (lhsT = o1T column windows, rhs = W2
rows), so its PSUM output is already row-major [128 tok, 512 feat] -- no PE
transposes, and the residual + RMSNorm epilogue works directly on token rows.
The kernel-2 causal conv's shift-add is absorbed as two accumulated matmul
windows in both layers. conv2_bias is folded into the residual on the host
(xrb2 = x + b2); ln_weight is applied exactly on the host.

Matmuls run in bf16 (fp32 PSUM accumulation); epilogue in fp32.
"""

import os

import numpy as np
import ml_dtypes

BS, L, D, CACHE = 8, 2048, 2048, 64
T = BS * L
H = D // 2          # 1024
EPS = 1e-6
NCORES = 8
BLK = 512           # token block (= one PSUM bank of fp32)
NBLK = L // BLK     # 4
KT1 = D // 128      # 16 contraction tiles, layer 1
KT2 = H // 128      # 8 contraction tiles, layer 2
QT1 = H // 128      # 8 output-feature tiles, layer 1 (per half)
TT = L // 128       # 16 token tiles, layer 2

TRACE = bool(int(os.environ.get("BASS_KERNEL_TRACE", "0")))
LAST_EXEC_NS = None
LAST_RESULTS = None

_NC_CACHE = {}


def _build_bass():
    if "nc" in _NC_CACHE:
        return _NC_CACHE["nc"]

    import concourse.bacc as bacc
    import concourse.tile as tile
    import concourse.mybir as mybir

    fp32 = mybir.dt.float32
    bf16 = mybir.dt.bfloat16
    Act = mybir.ActivationFunctionType

    nc = bacc.Bacc("TRN2", target_bir_lowering=False)

    xt1 = nc.declare_dram_parameter("xt1", [D, L + 1], bf16, isOutput=False)
    xrb2 = nc.declare_dram_parameter("xrb2", [L, D], fp32, isOutput=False)
    c2 = nc.declare_dram_parameter("c2", [H, 1], bf16, isOutput=False)
    w1 = nc.declare_dram_parameter("w1", [D, D], bf16, isOutput=False)
    w2 = nc.declare_dram_parameter("w2", [H, 2 * D], bf16, isOutput=False)
    b1 = nc.declare_dram_parameter("b1", [H, 1], fp32, isOutput=False)
    out = nc.declare_dram_parameter("out", [L, D], fp32, isOutput=True)

    with tile.TileContext(nc) as tc, \
            tc.tile_pool(name="w1p", bufs=1) as w1p, \
            tc.tile_pool(name="w2p", bufs=1) as w2p, \
            tc.tile_pool(name="xt2p", bufs=1) as xt2p, \
            tc.tile_pool(name="x1p", bufs=1) as x1p, \
            tc.tile_pool(name="xrp", bufs=1) as xrp, \
            tc.tile_pool(name="rowp", bufs=2) as rowp, \
            tc.tile_pool(name="tmp", bufs=2) as tmp, \
            tc.tile_pool(name="const", bufs=1) as const, \
            tc.tile_pool(name="ps", bufs=1, space="PSUM") as psp:

        # PE clock warmup input: memset first so the dummy matmuls can start
        # as early as possible (the tensor engine ramps 0.65->1.2->2.4 GHz
        # with ~3us of sustained work; the first real matmul can't start
        # before its DMAs land at ~3.6us).
        n_warm = int(os.environ.get("BASS_WARMUP_MM", "10"))
        warm_free = int(os.environ.get("BASS_WARMUP_FREE", "256"))
        dum = const.tile([128, BLK], bf16)
        nc.vector.memset(dum[:, 0:256], 0.0)

        # ------------- startup DMA stream (issue order == transfer order) ----
        # x1 tile (b0,k0) first, then W1 pair 0 in half-row pieces so the first
        # matmul unblocks after ~1.2 MB of DMA.
        x1k = []
        for k in range(KT1):
            x1k.append(x1p.tile([128, BLK + 1], bf16, tag=f"x1_{k}",
                                name=f"x1_0_{k}"))
        w1t = []
        for j in range(KT1 // 2):
            w1t.append(w1p.tile([128, 2, D], bf16, tag=f"w1_{j}",
                                name=f"w1_{j}"))

        # x1_0 goes through the Pool/SWDGE path: its descriptor generation
        # runs in parallel with the HWDGE generation of the w1 pieces, so the
        # first matmul's two dependencies pipeline instead of serializing.
        if os.environ.get("BASS_X1_SWDGE", "1") == "1":
            nc.gpsimd.dma_start(out=x1k[0], in_=xt1[0:128, 0:BLK + 1])
        else:
            nc.sync.dma_start(out=x1k[0], in_=xt1[0:128, 0:BLK + 1])
        nc.sync.dma_start(out=w1t[0][:, 0, 0:BLK], in_=w1[0:128, 0:BLK])
        nc.sync.dma_start(out=w1t[0][:, 0, BLK:D], in_=w1[0:128, BLK:D])
        nc.sync.dma_start(out=x1k[1], in_=xt1[128:256, 0:BLK + 1])
        nc.sync.dma_start(out=w1t[0][:, 1, :], in_=w1[128:256, :])
        nc.sync.dma_start(out=x1k[2], in_=xt1[256:384, 0:BLK + 1])
        for j in range(1, KT1 // 2):
            for kk in range(2):
                nc.sync.dma_start(
                    out=w1t[j][:, kk, :],
                    in_=w1[(2 * j + kk) * 128:(2 * j + kk + 1) * 128, :])
            for k in (2 * j + 1, 2 * j + 2):
                if k < KT1:
                    nc.sync.dma_start(
                        out=x1k[k], in_=xt1[k * 128:(k + 1) * 128, 0:BLK + 1])

        b1sb = const.tile([128, QT1, 1], fp32)
        epssb = const.tile([128, 1], fp32)
        xt2sb = xt2p.tile([128, KT2, L + 1], bf16)
        sqdump = const.tile([128, BLK], fp32)
        nc.sync.dma_start(out=b1sb, in_=b1.rearrange("(q p) o -> p q o", p=128))
        nc.sync.dma_start(
            out=xt2sb[:, :, 0:1], in_=c2.rearrange("(k p) o -> p k o", p=128))
        nc.vector.memset(epssb, EPS)

        # PE clock warmup: throwaway matmuls on the memset tile while the
        # first weight/activation DMAs are in flight, so the real matmuls
        # start at full clock.  Results go to a PSUM bank that the first real
        # accumulation group overwrites (start=True).
        if n_warm:
            wps = psp.tile([128, BLK], fp32, tag="ps0", name="ps_warm")
            for i in range(n_warm):
                nc.tensor.matmul(
                    wps[:, 0:warm_free], lhsT=dum[:, 0:128],
                    rhs=dum[:, 0:warm_free],
                    start=(i == 0), stop=(i == n_warm - 1))

        w2t = []
        for k in range(KT2):
            w2t.append(w2p.tile([128, 2 * D], bf16, tag=f"w2_{k}",
                                name=f"w2_{k}"))
        xrt = []
        for j in range(4):
            xrt.append(xrp.tile([128, BLK], fp32, tag=f"xr_{j}",
                                name=f"xr_0_{j}"))

        # ---------------- Phase A: layer 1 -> xt2sb (o1T, bf16) --------------
        for b in range(NBLK):
            psA = [psp.tile([128, BLK], fp32, tag=f"ps{q}", name=f"psA_{b}_{q}")
                   for q in range(QT1)]
            for k in range(KT1):
                xk = x1k[k]
                for q in range(QT1):
                    nc.tensor.matmul(
                        psA[q], lhsT=w1t[k // 2][:, k % 2, q * 128:(q + 1) * 128],
                        rhs=xk[:, 0:BLK], start=(k == 0), stop=False)
                for q in range(QT1):
                    nc.tensor.matmul(
                        psA[q],
                        lhsT=w1t[k // 2][:, k % 2, H + q * 128:H + (q + 1) * 128],
                        rhs=xk[:, 1:BLK + 1], start=False, stop=(k == KT1 - 1))
                if b < NBLK - 1:
                    # refresh this k-slot for the next block (WAR dep on the
                    # 16 matmuls just issued -- already satisfied when the DMA
                    # reaches the head of the queue).
                    x1k[k] = x1p.tile([128, BLK + 1], bf16, tag=f"x1_{k}",
                                      name=f"x1_{b + 1}_{k}")
                    nc.sync.dma_start(
                        out=x1k[k],
                        in_=xt1[k * 128:(k + 1) * 128,
                                (b + 1) * BLK:(b + 1) * BLK + BLK + 1])
            for q in range(QT1):
                nc.scalar.activation(
                    out=xt2sb[:, q, 1 + b * BLK:1 + (b + 1) * BLK], in_=psA[q],
                    func=Act.Identity, bias=b1sb[:, q, :], scale=1.0)
            # stagger W2 loads across blocks 0..2 so they never gate phase B
            for k in {0: (0, 1, 2), 1: (3, 4, 5), 2: (6, 7)}.get(b, ()):
                nc.sync.dma_start(out=w2t[k], in_=w2[k * 128:(k + 1) * 128, :])
            if b == NBLK - 1:
                # first token-tile's residual chunks for phase B
                for j in range(4):
                    nc.sync.dma_start(
                        out=xrt[j], in_=xrb2[0:128, j * BLK:(j + 1) * BLK])

        # ---------- Phase B: layer 2 token-major + residual + RMSNorm --------
        for t in range(TT):
            t0 = t * 128
            last = t == TT - 1
            rowc = rowp.tile([128, D], fp32, tag="row", name=f"row_{t}")
            # final tile: the last feature chunk is only 128 wide so the
            # add/square on the rstd critical path after the very last matmul
            # is short.
            chunks = [(0, BLK), (BLK, BLK), (2 * BLK, BLK),
                      (3 * BLK, 384), (3 * BLK + 384, 128)] if last else \
                     [(0, BLK), (BLK, BLK), (2 * BLK, BLK), (3 * BLK, BLK)]
            acc = tmp.tile([128, len(chunks)], fp32, tag="acc", name=f"acc_{t}")
            for ci, (c0, cw) in enumerate(chunks):
                f4 = c0 // BLK
                ps = psp.tile([128, BLK], fp32, tag=f"ps{(t * 4 + ci) % 8}",
                              name=f"psB_{t}_{ci}")
                for k in range(KT2):
                    nc.tensor.matmul(
                        ps[:, 0:cw], lhsT=xt2sb[:, k, t0:t0 + 128],
                        rhs=w2t[k][:, c0:c0 + cw],
                        start=(k == 0), stop=False)
                for k in range(KT2):
                    nc.tensor.matmul(
                        ps[:, 0:cw], lhsT=xt2sb[:, k, t0 + 1:t0 + 129],
                        rhs=w2t[k][:, D + c0:D + c0 + cw],
                        start=False, stop=(k == KT2 - 1))
                sl = slice(c0, c0 + cw)
                nc.vector.tensor_add(out=rowc[:, sl], in0=ps[:, 0:cw],
                                     in1=xrt[f4][:, c0 - f4 * BLK:c0 - f4 * BLK + cw])
                if not last and ci < 4:
                    xrt[ci] = xrp.tile([128, BLK], fp32, tag=f"xr_{ci}",
                                       name=f"xr_{t + 1}_{ci}")
                    nc.sync.dma_start(
                        out=xrt[ci],
                        in_=xrb2[t0 + 128:t0 + 256, ci * BLK:(ci + 1) * BLK])
                if last and ci == len(chunks) - 1:
                    # final chunk's sum-of-squares on DVE: the whole rstd
                    # dependency chain (add -> square -> reduce) stays on one
                    # engine -- no cross-engine hops on the critical path.
                    nc.vector.affine_mul_reduce(
                        out=sqdump[:, 0:cw], accum_out=acc[:, ci:ci + 1],
                        in0=rowc[:, sl], in1=rowc[:, sl], scale=1.0, bias=0.0)
                else:
                    nc.scalar.activation(
                        out=sqdump[:, 0:cw], in_=rowc[:, sl],
                        func=Act.Square, accum_out=acc[:, ci:ci + 1])
            rstd = tmp.tile([128, 1], fp32, tag="rstd", name=f"rstd_{t}")
            nc.vector.tensor_reduce(
                out=rstd, in_=acc, axis=mybir.AxisListType.X,
                op=mybir.AluOpType.add)
            nc.scalar.activation(
                out=rstd, in_=rstd, func=Act.Sqrt, bias=epssb, scale=1.0 / D)
            nc.vector.reciprocal(out=rstd, in_=rstd)
            if last:
                # 4-way scale/store, DVE first, so the first store's
                # descriptors enter the (serialized) DMA path asap.
                for j in range(4):
                    sl = slice(j * BLK, (j + 1) * BLK)
                    if j % 2 == 0:
                        nc.vector.tensor_scalar_mul(
                            out=rowc[:, sl], in0=rowc[:, sl], scalar1=rstd)
                    else:
                        nc.scalar.activation(
                            out=rowc[:, sl], in_=rowc[:, sl],
                            func=Act.Identity, bias=0.0, scale=rstd)
                    nc.sync.dma_start(
                        out=out[t0:t0 + 128, sl], in_=rowc[:, sl])
            else:
                nc.scalar.activation(
                    out=rowc[:, 0:H], in_=rowc[:, 0:H],
                    func=Act.Identity, bias=0.0, scale=rstd)
                nc.vector.tensor_scalar_mul(
                    out=rowc[:, H:D], in0=rowc[:, H:D], scalar1=rstd)
                nc.sync.dma_start(out=out[t0:t0 + 128, 0:H], in_=rowc[:, 0:H])
                nc.sync.dma_start(out=out[t0:t0 + 128, H:D], in_=rowc[:, H:D])

    nc.finalize()
    _NC_CACHE["nc"] = nc
    return nc


def _np_reference(inputs, pre_lf_indexs, out_lf_indexs, input_lf_loc, out_lf_loc,
                  inputs_loc, outputs_loc, lf1_caches, lf2_caches,
                  conv1_weight, conv2_weight, conv1_bias, conv2_bias, ln_weight):
    """Generic numpy fallback (only used if the index structure is unexpected)."""
    def fused(x, cache, pre_idx, in_lf_loc, in_loc, out_loc, W):
        bs = pre_idx.shape[0]
        xt = np.zeros((x.shape[0] + bs, x.shape[1]), x.dtype)
        xt[in_loc] = x
        xt[in_lf_loc] = cache[pre_idx]
        c = xt @ W
        h = c.shape[1] // 2
        y = c[:-1, :h] + c[1:, h:]
        return y[out_loc]

    o1 = fused(inputs, lf1_caches, pre_lf_indexs, input_lf_loc,
               inputs_loc, outputs_loc, conv1_weight) + conv1_bias
    o2 = fused(o1, lf2_caches, pre_lf_indexs, input_lf_loc,
               inputs_loc, outputs_loc, conv2_weight) + conv2_bias
    o3 = o2 + inputs
    var = np.mean(o3 * o3, axis=-1, keepdims=True)
    return (o3 / np.sqrt(var + EPS) * ln_weight).astype(np.float32)


def kernel(**inputs):
    global LAST_EXEC_NS, LAST_RESULTS
    inp = {k: np.asarray(v) for k, v in inputs.items()}
    x = inp["inputs"].astype(np.float32, copy=False)
    lnw = inp["ln_weight"].astype(np.float32, copy=False)

    s = np.arange(BS, dtype=np.int64)
    j = np.arange(L, dtype=np.int64)
    structured = (
        np.array_equal(inp["inputs_loc"], (s[:, None] * (L + 1) + 1 + j[None, :]).reshape(-1))
        and np.array_equal(inp["outputs_loc"], (s[:, None] * (L + 1) + j[None, :]).reshape(-1))
        and np.array_equal(inp["input_lf_loc"], s * (L + 1))
    )
    if not structured:
        return _np_reference(**inp)

    from concourse.bass_utils import run_bass_kernel_spmd

    nc = _build_bass()

    bf16 = ml_dtypes.bfloat16
    pre_idx = inp["pre_lf_indexs"].astype(np.int64)
    w1b = np.ascontiguousarray(inp["conv1_weight"].astype(bf16))
    w2b = np.ascontiguousarray(inp["conv2_weight"].astype(bf16))
    b1f = np.ascontiguousarray(inp["conv1_bias"].astype(np.float32).reshape(H, 1))
    b2row = inp["conv2_bias"].astype(np.float32).reshape(1, D)

    in_maps = []
    for sq in range(BS):
        xs = x[sq * L:(sq + 1) * L]                       # [2048, 2048]
        a = np.empty((D, L + 1), np.float32)
        a[:, 0] = inp["lf1_caches"][pre_idx[sq]]
        a[:, 1:] = xs.T
        in_maps.append({
            "xt1": np.ascontiguousarray(a.astype(bf16)),
            "xrb2": np.ascontiguousarray(xs + b2row),
            "c2": np.ascontiguousarray(
                inp["lf2_caches"][pre_idx[sq]].astype(bf16).reshape(H, 1)),
            "w1": w1b,
            "w2": w2b,
            "b1": b1f,
        })

    res = run_bass_kernel_spmd(nc, in_maps, list(range(NCORES)), trace=TRACE)
    LAST_EXEC_NS = res.exec_time_ns
    LAST_RESULTS = res
    out = np.concatenate([res.results[i]["out"] for i in range(NCORES)], axis=0)
    if not np.all(lnw == 1.0):
        out = out * lnw[None, :]
    return out.astype(np.float32)


# revision 17
# speedup vs baseline: 1.1153x; 1.0009x over previous
"""Trainium2 Bass kernel for nn_LocalizedFiltering (fused cat-conv2d x2 + residual + RMSNorm).

Strategy: sequence-parallel across 8 NeuronCores (one sequence of 2048 tokens +
1 cache row per core) -- no collectives needed.

Layer 1 runs feature-on-partition (output o1T = [1024 feat, 2049 tok] bf16 in
SBUF); layer 2 runs token-on-partition (lhsT = o1T column windows, rhs = W2
rows), so its PSUM output is already row-major [128 tok, 512 feat] -- no PE
transposes, and the residual + RMSNorm epilogue works directly on token rows.
The kernel-2 causal conv's shift-add is absorbed as two accumulated matmul
windows in both layers. conv2_bias is folded into the residual on the host
(xrb2 = x + b2); ln_weight is applied exactly on the host.

Matmuls run in bf16 (fp32 PSUM accumulation); epilogue in fp32.
"""

import os

import numpy as np
import ml_dtypes

BS, L, D, CACHE = 8, 2048, 2048, 64
T = BS * L
H = D // 2          # 1024
EPS = 1e-6
NCORES = 8
BLK = 512           # token block (= one PSUM bank of fp32)
NBLK = L // BLK     # 4
KT1 = D // 128      # 16 contraction tiles, layer 1
KT2 = H // 128      # 8 contraction tiles, layer 2
QT1 = H // 128      # 8 output-feature tiles, layer 1 (per half)
TT = L // 128       # 16 token tiles, layer 2

TRACE = bool(int(os.environ.get("BASS_KERNEL_TRACE", "0")))
LAST_EXEC_NS = None
LAST_RESULTS = None

_NC_CACHE = {}


def _build_bass():
    if "nc" in _NC_CACHE:
        return _NC_CACHE["nc"]

    import concourse.bacc as bacc
    import concourse.tile as tile
    import concourse.mybir as mybir

    fp32 = mybir.dt.float32
    bf16 = mybir.dt.bfloat16
    Act = mybir.ActivationFunctionType

    nc = bacc.Bacc("TRN2", target_bir_lowering=False)

    xt1 = nc.declare_dram_parameter("xt1", [D, L + 1], bf16, isOutput=False)
    xrb2 = nc.declare_dram_parameter("xrb2", [L, D], fp32, isOutput=False)
    c2 = nc.declare_dram_parameter("c2", [H, 1], bf16, isOutput=False)
    w1 = nc.declare_dram_parameter("w1", [D, D], bf16, isOutput=False)
    w2 = nc.declare_dram_parameter("w2", [H, 2 * D], bf16, isOutput=False)
    b1 = nc.declare_dram_parameter("b1", [H, 1], fp32, isOutput=False)
    out = nc.declare_dram_parameter("out", [L, D], fp32, isOutput=True)

    with tile.TileContext(nc) as tc, \
            tc.tile_pool(name="w1p", bufs=1) as w1p, \
            tc.tile_pool(name="w2p", bufs=1) as w2p, \
            tc.tile_pool(name="xt2p", bufs=1) as xt2p, \
            tc.tile_pool(name="x1p", bufs=1) as x1p, \
            tc.tile_pool(name="xrp", bufs=1) as xrp, \
            tc.tile_pool(name="rowp", bufs=2) as rowp, \
            tc.tile_pool(name="tmp", bufs=2) as tmp, \
            tc.tile_pool(name="const", bufs=1) as const, \
            tc.tile_pool(name="ps", bufs=1, space="PSUM") as psp:

        # PE clock warmup input: memset first so the dummy matmuls can start
        # as early as possible (the tensor engine ramps 0.65->1.2->2.4 GHz
        # with ~3us of sustained work; the first real matmul can't start
        # before its DMAs land at ~3.6us).
        n_warm = int(os.environ.get("BASS_WARMUP_MM", "10"))
        warm_free = int(os.environ.get("BASS_WARMUP_FREE", "256"))
        dum = const.tile([128, BLK], bf16)
        nc.vector.memset(dum[:, 0:256], 0.0)

        # ------------- startup DMA stream (issue order == transfer order) ----
        # x1 tile (b0,k0) first, then W1 pair 0 in half-row pieces so the first
        # matmul unblocks after ~1.2 MB of DMA.
        x1k = []
        for k in range(KT1):
            x1k.append(x1p.tile([128, BLK + 1], bf16, tag=f"x1_{k}",
                                name=f"x1_0_{k}"))
        w1t = []
        for j in range(KT1 // 2):
            w1t.append(w1p.tile([128, 2, D], bf16, tag=f"w1_{j}",
                                name=f"w1_{j}"))

        # x1_0 goes through the Pool/SWDGE path: its descriptor generation
        # runs in parallel with the HWDGE generation of the w1 pieces, so the
        # first matmul's two dependencies pipeline instead of serializing.
        if os.environ.get("BASS_X1_SWDGE", "1") == "1":
            nc.gpsimd.dma_start(out=x1k[0], in_=xt1[0:128, 0:BLK + 1])
        else:
            nc.sync.dma_start(out=x1k[0], in_=xt1[0:128, 0:BLK + 1])
        for p in range(4):
            nc.sync.dma_start(out=w1t[0][:, 0, p * BLK:(p + 1) * BLK],
                              in_=w1[0:128, p * BLK:(p + 1) * BLK])
        nc.sync.dma_start(out=x1k[1], in_=xt1[128:256, 0:BLK + 1])
        nc.sync.dma_start(out=w1t[0][:, 1, :], in_=w1[128:256, :])
        nc.sync.dma_start(out=x1k[2], in_=xt1[256:384, 0:BLK + 1])
        for j in range(1, KT1 // 2):
            for kk in range(2):
                nc.sync.dma_start(
                    out=w1t[j][:, kk, :],
                    in_=w1[(2 * j + kk) * 128:(2 * j + kk + 1) * 128, :])
            for k in (2 * j + 1, 2 * j + 2):
                if k < KT1:
                    nc.sync.dma_start(
                        out=x1k[k], in_=xt1[k * 128:(k + 1) * 128, 0:BLK + 1])

        b1sb = const.tile([128, QT1, 1], fp32)
        epssb = const.tile([128, 1], fp32)
        xt2sb = xt2p.tile([128, KT2, L + 1], bf16)
        sqdump = const.tile([128, BLK], fp32)
        nc.sync.dma_start(out=b1sb, in_=b1.rearrange("(q p) o -> p q o", p=128))
        nc.sync.dma_start(
            out=xt2sb[:, :, 0:1], in_=c2.rearrange("(k p) o -> p k o", p=128))
        nc.vector.memset(epssb, EPS)

        # PE clock warmup: throwaway matmuls on the memset tile while the
        # first weight/activation DMAs are in flight, so the real matmuls
        # start at full clock.  Results go to a PSUM bank that the first real
        # accumulation group overwrites (start=True).
        if n_warm:
            wps = psp.tile([128, BLK], fp32, tag="ps0", name="ps_warm")
            for i in range(n_warm):
                nc.tensor.matmul(
                    wps[:, 0:warm_free], lhsT=dum[:, 0:128],
                    rhs=dum[:, 0:warm_free],
                    start=(i == 0), stop=(i == n_warm - 1))

        w2t = []
        for k in range(KT2):
            w2t.append(w2p.tile([128, 2 * D], bf16, tag=f"w2_{k}",
                                name=f"w2_{k}"))
        xrt = []
        for j in range(4):
            xrt.append(xrp.tile([128, BLK], fp32, tag=f"xr_{j}",
                                name=f"xr_0_{j}"))

        # ---------------- Phase A: layer 1 -> xt2sb (o1T, bf16) --------------
        for b in range(NBLK):
            psA = [psp.tile([128, BLK], fp32, tag=f"ps{q}", name=f"psA_{b}_{q}")
                   for q in range(QT1)]
            for k in range(KT1):
                xk = x1k[k]
                for q in range(QT1):
                    nc.tensor.matmul(
                        psA[q], lhsT=w1t[k // 2][:, k % 2, q * 128:(q + 1) * 128],
                        rhs=xk[:, 0:BLK], start=(k == 0), stop=False)
                for q in range(QT1):
                    nc.tensor.matmul(
                        psA[q],
                        lhsT=w1t[k // 2][:, k % 2, H + q * 128:H + (q + 1) * 128],
                        rhs=xk[:, 1:BLK + 1], start=False, stop=(k == KT1 - 1))
                if b < NBLK - 1:
                    # refresh this k-slot for the next block (WAR dep on the
                    # 16 matmuls just issued -- already satisfied when the DMA
                    # reaches the head of the queue).
                    x1k[k] = x1p.tile([128, BLK + 1], bf16, tag=f"x1_{k}",
                                      name=f"x1_{b + 1}_{k}")
                    nc.sync.dma_start(
                        out=x1k[k],
                        in_=xt1[k * 128:(k + 1) * 128,
                                (b + 1) * BLK:(b + 1) * BLK + BLK + 1])
            # drain PSUM -> xt2sb on two engines so the next block's first
            # matmuls (WAR on these banks) aren't gated by one engine's
            # serial drain ladder.
            for q in range(QT1):
                if q % 2 == 0:
                    nc.scalar.activation(
                        out=xt2sb[:, q, 1 + b * BLK:1 + (b + 1) * BLK],
                        in_=psA[q],
                        func=Act.Identity, bias=b1sb[:, q, :], scale=1.0)
                else:
                    nc.vector.tensor_scalar_add(
                        out=xt2sb[:, q, 1 + b * BLK:1 + (b + 1) * BLK],
                        in0=psA[q], scalar1=b1sb[:, q, :])
            # stagger W2 loads across blocks 0..2 so they never gate phase B
            for k in {0: (0, 1, 2), 1: (3, 4, 5), 2: (6, 7)}.get(b, ()):
                nc.sync.dma_start(out=w2t[k], in_=w2[k * 128:(k + 1) * 128, :])
            if b == NBLK - 1:
                # first token-tile's residual chunks for phase B
                for j in range(4):
                    nc.sync.dma_start(
                        out=xrt[j], in_=xrb2[0:128, j * BLK:(j + 1) * BLK])

        # ---------- Phase B: layer 2 token-major + residual + RMSNorm --------
        for t in range(TT):
            t0 = t * 128
            last = t == TT - 1
            rowc = rowp.tile([128, D], fp32, tag="row", name=f"row_{t}")
            # final tile: the last feature chunk is only 128 wide so the
            # add/square on the rstd critical path after the very last matmul
            # is short.
            chunks = [(0, BLK), (BLK, BLK), (2 * BLK, BLK),
                      (3 * BLK, 384), (3 * BLK + 384, 128)] if last else \
                     [(0, BLK), (BLK, BLK), (2 * BLK, BLK), (3 * BLK, BLK)]
            acc = tmp.tile([128, len(chunks)], fp32, tag="acc", name=f"acc_{t}")
            for ci, (c0, cw) in enumerate(chunks):
                f4 = c0 // BLK
                ps = psp.tile([128, BLK], fp32, tag=f"ps{(t * 4 + ci) % 8}",
                              name=f"psB_{t}_{ci}")
                for k in range(KT2):
                    nc.tensor.matmul(
                        ps[:, 0:cw], lhsT=xt2sb[:, k, t0:t0 + 128],
                        rhs=w2t[k][:, c0:c0 + cw],
                        start=(k == 0), stop=False)
                for k in range(KT2):
                    nc.tensor.matmul(
                        ps[:, 0:cw], lhsT=xt2sb[:, k, t0 + 1:t0 + 129],
                        rhs=w2t[k][:, D + c0:D + c0 + cw],
                        start=False, stop=(k == KT2 - 1))
                sl = slice(c0, c0 + cw)
                nc.vector.tensor_add(out=rowc[:, sl], in0=ps[:, 0:cw],
                                     in1=xrt[f4][:, c0 - f4 * BLK:c0 - f4 * BLK + cw])
                if not last and ci < 4:
                    xrt[ci] = xrp.tile([128, BLK], fp32, tag=f"xr_{ci}",
                                       name=f"xr_{t + 1}_{ci}")
                    nc.sync.dma_start(
                        out=xrt[ci],
                        in_=xrb2[t0 + 128:t0 + 256, ci * BLK:(ci + 1) * BLK])
                if last and ci == len(chunks) - 1:
                    # final chunk's sum-of-squares on DVE: the whole rstd
                    # dependency chain (add -> square -> reduce) stays on one
                    # engine -- no cross-engine hops on the critical path.
                    nc.vector.affine_mul_reduce(
                        out=sqdump[:, 0:cw], accum_out=acc[:, ci:ci + 1],
                        in0=rowc[:, sl], in1=rowc[:, sl], scale=1.0, bias=0.0)
                else:
                    nc.scalar.activation(
                        out=sqdump[:, 0:cw], in_=rowc[:, sl],
                        func=Act.Square, accum_out=acc[:, ci:ci + 1])
            rstd = tmp.tile([128, 1], fp32, tag="rstd", name=f"rstd_{t}")
            nc.vector.tensor_reduce(
                out=rstd, in_=acc, axis=mybir.AxisListType.X,
                op=mybir.AluOpType.add)
            nc.scalar.activation(
                out=rstd, in_=rstd, func=Act.Sqrt, bias=epssb, scale=1.0 / D)
            nc.vector.reciprocal(out=rstd, in_=rstd)
            if last:
                # 4-way scale/store, DVE first, so the first store's
                # descriptors enter the (serialized) DMA path asap.
                for j in range(4):
                    sl = slice(j * BLK, (j + 1) * BLK)
                    if j % 2 == 0:
                        nc.vector.tensor_scalar_mul(
                            out=rowc[:, sl], in0=rowc[:, sl], scalar1=rstd)
                    else:
                        nc.scalar.activation(
                            out=rowc[:, sl], in_=rowc[:, sl],
                            func=Act.Identity, bias=0.0, scale=rstd)
                    nc.sync.dma_start(
                        out=out[t0:t0 + 128, sl], in_=rowc[:, sl])
            else:
                nc.scalar.activation(
                    out=rowc[:, 0:H], in_=rowc[:, 0:H],
                    func=Act.Identity, bias=0.0, scale=rstd)
                nc.vector.tensor_scalar_mul(
                    out=rowc[:, H:D], in0=rowc[:, H:D], scalar1=rstd)
                nc.sync.dma_start(out=out[t0:t0 + 128, 0:H], in_=rowc[:, 0:H])
                nc.sync.dma_start(out=out[t0:t0 + 128, H:D], in_=rowc[:, H:D])

    nc.finalize()
    _NC_CACHE["nc"] = nc
    return nc


def _np_reference(inputs, pre_lf_indexs, out_lf_indexs, input_lf_loc, out_lf_loc,
                  inputs_loc, outputs_loc, lf1_caches, lf2_caches,
                  conv1_weight, conv2_weight, conv1_bias, conv2_bias, ln_weight):
    """Generic numpy fallback (only used if the index structure is unexpected)."""
    def fused(x, cache, pre_idx, in_lf_loc, in_loc, out_loc, W):
        bs = pre_idx.shape[0]
        xt = np.zeros((x.shape[0] + bs, x.shape[1]), x.dtype)
        xt[in_loc] = x
        xt[in_lf_loc] = cache[pre_idx]
        c = xt @ W
        h = c.shape[1] // 2
        y = c[:-1, :h] + c[1:, h:]
        return y[out_loc]

    o1 = fused(inputs, lf1_caches, pre_lf_indexs, input_lf_loc,
               inputs_loc, outputs_loc, conv1_weight) + conv1_bias
    o2 = fused(o1, lf2_caches, pre_lf_indexs, input_lf_loc,
               inputs_loc, outputs_loc, conv2_weight) + conv2_bias
    o3 = o2 + inputs
    var = np.mean(o3 * o3, axis=-1, keepdims=True)
    return (o3 / np.sqrt(var + EPS) * ln_weight).astype(np.float32)


def kernel(**inputs):
    global LAST_EXEC_NS, LAST_RESULTS
    inp = {k: np.asarray(v) for k, v in inputs.items()}
    x = inp["inputs"].astype(np.float32, copy=False)
    lnw = inp["ln_weight"].astype(np.float32, copy=False)

    s = np.arange(BS, dtype=np.int64)
    j = np.arange(L, dtype=np.int64)
    structured = (
        np.array_equal(inp["inputs_loc"], (s[:, None] * (L + 1) + 1 + j[None, :]).reshape(-1))
        and np.array_equal(inp["outputs_loc"], (s[:, None] * (L + 1) + j[None, :]).reshape(-1))
        and np.array_equal(inp["input_lf_loc"], s * (L + 1))
    )
    if not structured:
        return _np_reference(**inp)

    from concourse.bass_utils import run_bass_kernel_spmd

    nc = _build_bass()

    bf16 = ml_dtypes.bfloat16
    pre_idx = inp["pre_lf_indexs"].astype(np.int64)
    w1b = np.ascontiguousarray(inp["conv1_weight"].astype(bf16))
    w2b = np.ascontiguousarray(inp["conv2_weight"].astype(bf16))
    b1f = np.ascontiguousarray(inp["conv1_bias"].astype(np.float32).reshape(H, 1))
    b2row = inp["conv2_bias"].astype(np.float32).reshape(1, D)

    in_maps = []
    for sq in range(BS):
        xs = x[sq * L:(sq + 1) * L]                       # [2048, 2048]
        a = np.empty((D, L + 1), np.float32)
        a[:, 0] = inp["lf1_caches"][pre_idx[sq]]
        a[:, 1:] = xs.T
        in_maps.append({
            "xt1": np.ascontiguousarray(a.astype(bf16)),
            "xrb2": np.ascontiguousarray(xs + b2row),
            "c2": np.ascontiguousarray(
                inp["lf2_caches"][pre_idx[sq]].astype(bf16).reshape(H, 1)),
            "w1": w1b,
            "w2": w2b,
            "b1": b1f,
        })

    res = run_bass_kernel_spmd(nc, in_maps, list(range(NCORES)), trace=TRACE)
    LAST_EXEC_NS = res.exec_time_ns
    LAST_RESULTS = res
    out = np.concatenate([res.results[i]["out"] for i in range(NCORES)], axis=0)
    if not np.all(lnw == 1.0):
        out = out * lnw[None, :]
    return out.astype(np.float32)
